# revision 50
# baseline (speedup 1.0000x reference)
"""Causal self-attention (B=2, T=2048, C=1024, H=16) on 8 TRN2 NeuronCores.

Sharding: core = b * 4 + g  ->  batch b, head-group g (4 heads of 64 dims).
Each core computes the qkv projection for its 4 heads, causal attention, and
a partial c_proj contribution; the host sums the 4 partials per batch.

v2 (vs. the fp32r baseline):
  - x is transposed AND cast to bf16 on the host; all matmul operands are
    bf16 (PSUM accumulation stays fp32).  This removes all 128 PE
    transposes + their LDWEIGHTS + the x^T PSUM->SBUF copies.
  - The whole x^T [1024, 2048] lives in SBUF (32 KiB/partition in bf16);
    it is loaded with 8 large DMAs on the Sync queue while the weights load
    in parallel on the Activation queue, so the PE starts ~2 us in.
  - exp() writes P directly as bf16; AV and c_proj consume bf16.
  - softmax denominators use reciprocal_approx_fast (~5x cheaper on DVE).
  - y stores go out on the Activation HWDGE queue; PSUM->SBUF copies for y
    are split between DVE and GpSimd to keep DVE off the critical path.
  - the last q-slice runs pair 0's tail (recip/normalize) under pair 1's
    attention to shrink the end-of-kernel drain.

Attention per (pair, q-slice): S^T = K^T q-block (row-packed head pairs),
one exp per k-tile over both heads via a 3D AP, GPSIMD affine_select zeroes
the causal triangle, AV accumulates O^T[65,512] whose row 64 is the softmax
denominator (ones column in V).  Normalization is deferred off the critical
path; O^T feeds c_proj directly as the stationary operand.
"""

import sys

sys.path.insert(0, "/opt/trn_rl_repo")

import ml_dtypes
import numpy as np

import concourse.bass as bass
import concourse.mybir as mybir
import concourse.tile as tile
from concourse import bacc
from concourse.bass_utils import run_bass_kernel_spmd

B, T, C = 2, 2048, 1024
H = 16          # total heads
HC = 4          # heads per core
D = 64          # head dim
N_CORES = 8
TT = T // 128   # 16 token tiles
CK = C // 128   # 8 input-feature tiles
QS = T // 512   # 4 q-slices
PAIRS = 2       # head pairs per core

F32 = mybir.dt.float32
F32R = mybir.dt.float32r
BF16 = mybir.dt.bfloat16
EXPF = mybir.ActivationFunctionType.Exp
GE = mybir.AluOpType.is_ge


def build_program():
    nc = bacc.Bacc("TRN2", target_bir_lowering=False, debug=False,
                   num_devices=N_CORES)
    xT = nc.dram_tensor("xT", [C, T], BF16, kind="ExternalInput").ap()
    wqkv = nc.dram_tensor("wqkv", [C, 3 * HC * D], BF16,
                          kind="ExternalInput").ap()
    wp = nc.dram_tensor("wp", [HC * D, C], BF16, kind="ExternalInput").ap()
    ones16 = nc.dram_tensor("ones16", [128, 128], BF16,
                            kind="ExternalInput").ap()
    onesr = nc.dram_tensor("onesr", [33, 128], F32R,
                           kind="ExternalInput").ap()
    yout = nc.dram_tensor("y", [T, C], F32, kind="ExternalOutput").ap()

    with tile.TileContext(nc) as tc:
        build_kernel(nc, tc, xT, wqkv, wp, ones16, onesr, yout)
    nc.compile()
    return nc


def head2(ap_2d, o, width):
    """[128, 1024] tile viewed as [128, 2 heads, width] starting at col o."""
    return ap_2d.rearrange("p (h c) -> p h c", h=2)[:, :, o:o + width]


class Weave:
    """Round-robin emitter: interleaves closures from several work lists so
    each engine's in-order stream alternates between independent chains."""

    def __init__(self):
        self.lists = []

    def add(self, ops):
        if ops:
            self.lists.append(list(ops))

    def run(self):
        lists = [l for l in self.lists if l]
        total = sum(len(l) for l in lists)
        emitted = 0
        idx = [0] * len(lists)
        while emitted < total:
            best, bfrac = None, None
            for n, l in enumerate(lists):
                if idx[n] < len(l):
                    frac = idx[n] / len(l)
                    if bfrac is None or frac < bfrac:
                        best, bfrac = n, frac
            lists[best][idx[best]]()
            idx[best] += 1
            emitted += 1
        self.lists = []


def build_kernel(nc, tc, xT, wqkv, wp, ones16, onesr, yout):
    from contextlib import ExitStack

    ctx = ExitStack()
    with ctx:
        const = ctx.enter_context(tc.tile_pool(name="const", bufs=1))
        # x^T split into disjoint tiles per k: tokens 0-511 (slice 0) and
        # tokens 512-2047 (slices 1-3).  The slice-0 tiles load first on
        # the Sync HWDGE queue (~1 MB, ~3 us) so slice 0's qkv projection
        # starts almost immediately; the rest streams in behind them.
        # Separate tiles keep the DMA-completion dependency per slice-0
        # read off the big transfers.
        xt0_sb = []
        for k in range(CK):
            t = const.tile([128, 512], BF16, tag=f"xT0_{k}", name=f"xT0_{k}")
            nc.sync.dma_start(t[:], xT[k * 128:(k + 1) * 128, 0:512])
            xt0_sb.append(t)
        xt_sb = []
        for k in range(CK):
            t = const.tile([128, T - 512], BF16, tag=f"xT{k}",
                           name=f"xT{k}")
            nc.sync.dma_start(t[:], xT[k * 128:(k + 1) * 128, 512:T])
            xt_sb.append(t)
        # Weights + small constants in parallel on the Act HWDGE queue.
        # Q/K columns first (the slice-0 S-chain needs them immediately);
        # V columns follow behind, in separate tiles so the Q/K readers
        # don't wait on the V transfers.
        wqk_sb = []
        for k in range(CK):
            t = const.tile([128, 2 * HC * D], BF16, tag=f"wqk{k}",
                           name=f"wqk{k}")
            nc.scalar.dma_start(t[:],
                                wqkv[k * 128:(k + 1) * 128, 0:2 * HC * D])
            wqk_sb.append(t)
        wv_sb = []
        for k in range(CK):
            t = const.tile([128, HC * D], BF16, tag=f"wv{k}", name=f"wv{k}")
            nc.scalar.dma_start(
                t[:], wqkv[k * 128:(k + 1) * 128, 2 * HC * D:3 * HC * D])
            wv_sb.append(t)
        wp_sb = []
        for p in range(2):
            t = const.tile([128, C], BF16, tag=f"wp{p}", name=f"wp{p}")
            nc.scalar.dma_start(t[:], wp[p * 128:(p + 1) * 128, :])
            wp_sb.append(t)
        ones = const.tile([128, 128], BF16, tag="ones", name="ones")
        nc.scalar.dma_start(ones[:], ones16[:])
        blk2 = const.tile([33, 128], F32R, tag="blk2", name="blk2")
        nc.scalar.dma_start(blk2[:], onesr[:])

        big = ctx.enter_context(tc.tile_pool(name="big", bufs=1))
        KT = [big.tile([128, T], BF16, tag=f"KT{p}", name=f"KT{p}")
              for p in range(PAIRS)]
        VP = [big.tile([128, HC * (D + 1)], BF16, tag=f"VP{i}",
                       name=f"VP{i}") for i in range(TT)]
        for i in range(TT):
            vp3 = VP[i][:].rearrange("p (h c) -> p h c", c=D + 1)
            nc.vector.tensor_copy(
                vp3[:, :, D:D + 1],
                ones[:, 0:HC].rearrange("p (h c) -> p h c", c=1))
        # per-slice rotating tiles (live for ~one pipeline step each)
        qtp = ctx.enter_context(tc.tile_pool(name="QTs", bufs=2))
        otp = ctx.enter_context(tc.tile_pool(name="OTs", bufs=2))
        qt_slice = {}   # ts -> [QT tile per pair]  [128 (2hd x 64d), 512]
        ot_slice = {}   # qs -> [O^T tile per pair] [128 (2hd x 64d), 512]
        # fp32 spill of the early (off-diagonal) AV accumulation; reused
        # across q-slices.
        oacc = [[big.tile([65, 512], F32, tag=f"oa{p}{hp}",
                          name=f"oa{p}{hp}") for hp in range(2)]
                for p in range(PAIRS)]
        # denominator slots: a [2,512] tile per (q-slice, pair), row hp;
        # one reciprocal + one broadcast matmul serves both heads.
        rsg = [[big.tile([33, 512], F32, tag=f"rsg{q}_{p}",
                         name=f"rsg{q}_{p}") for p in range(PAIRS)]
               for q in range(QS)]
        recg = [[big.tile([33, 512], F32, tag=f"recg{q}_{p}",
                          name=f"recg{q}_{p}") for p in range(PAIRS)]
                for q in range(QS)]
        # fill once so the reciprocal of the unused rows (1..31) stays
        # finite; rows 0/32 are overwritten with real denominators.
        for q in range(QS):
            for p in range(PAIRS):
                nc.gpsimd.memset(rsg[q][p][:], 1.0)

        ptpool = ctx.enter_context(tc.tile_pool(name="pt", bufs=6))
        nrm = ctx.enter_context(tc.tile_pool(name="nrm", bufs=4))
        ypool = ctx.enter_context(tc.tile_pool(name="ysb", bufs=3))
        # PSUM budget (8 banks): s 2x2 + av 2x1 + A-phase/proj/rb 2x1
        sps = ctx.enter_context(tc.tile_pool(name="sps", bufs=2,
                                             space="PSUM"))
        avps = ctx.enter_context(tc.tile_pool(name="avps", bufs=1,
                                              space="PSUM"))
        aps = ctx.enter_context(tc.tile_pool(name="aps", bufs=2,
                                             space="PSUM"))

        def emit_proj_ops(ts):
            """A-phase for slice ts: V and Q/K projections straight from
            the resident bf16 x^T tiles (no loads, no transposes).  Slice 0
            reads the small early-loaded x^T copy."""
            ops = []
            xs = xt0_sb if ts == 0 else xt_sb
            xo = 0 if ts == 0 else (ts - 1) * 512

            def v_group(j):
                def f():
                    i = ts * 4 + j
                    ps = aps.tile([128, HC * D], F32, tag="a", name="a")
                    for k in range(CK):
                        nc.tensor.matmul(
                            ps[:],
                            xs[k][:, j * 128 + xo:(j + 1) * 128 + xo],
                            wv_sb[k][:],
                            start=(k == 0), stop=(k == CK - 1))
                    vp3 = VP[i][:].rearrange("p (h c) -> p h c", c=D + 1)
                    nc.vector.tensor_copy(
                        vp3[:, :, 0:D],
                        ps[:].rearrange("p (h c) -> p h c", c=D))
                return f

            def qk_group(ft):
                def f():
                    ps = aps.tile([128, 512], F32, tag="a", name="a")
                    for k in range(CK):
                        nc.tensor.matmul(
                            ps[:],
                            wqk_sb[k][:, ft * 128:(ft + 1) * 128],
                            xs[k][:, xo:xo + 512],
                            start=(k == 0), stop=(k == CK - 1))
                    if ft < 2:
                        qt = qtp.tile([128, 512], BF16, tag=f"QT{ft}",
                                      name=f"QT{ft}")
                        qt_slice.setdefault(ts, [None, None])[ft] = qt
                        nc.vector.tensor_copy(qt[:], ps[:])
                    else:
                        nc.vector.tensor_copy(
                            KT[ft - 2][:, ts * 512:(ts + 1) * 512], ps[:])
                return f

            # Q/K first: the attention S-chain for this slice (and the
            # early segment of the next) depends on QT/KT, not on V.
            for ft in range(4):
                ops.append(qk_group(ft))
            for j in range(4):
                ops.append(v_group(j))
            return ops

        def emit_att_pair_ops(qs, pair, ks=0, ke=None, seg="full"):
            """B-phase: attention for q-slice qs, one pair, k-tiles
            [ks, ke); AV lagged one k-tile behind S so the PE rarely waits
            on a just-issued exp.  seg='early' accumulates k-tiles that are
            ready one pipeline step ahead and spills the partial O^T to
            SBUF; seg='late' finishes the diagonal tiles and merges the
            spill; seg='full' is the unsplit path (q-slice 0)."""
            ops = []
            if ke is None:
                ke = 4 * qs + 4
            avs = [None, None]
            pts = {}

            def start_pair():
                for hp in range(2):
                    avs[hp] = avps.tile([65, 512], F32, tag=f"av{hp}",
                                        name=f"av{hp}")
                if seg != "early" and \
                        ot_slice.setdefault(qs, [None, None])[pair] is None:
                    ot_slice[qs][pair] = otp.tile(
                        [128, 512], BF16, tag=f"OT{pair}",
                        name=f"OT{pair}")

            def s_exp(ki):
                def f():
                    o = max(0, 128 * ki - 512 * qs)
                    s = sps.tile([128, 1024], F32, tag="s", name="s")
                    for hp in range(2):
                        nc.tensor.matmul(
                            s[:, hp * 512 + o:hp * 512 + 512],
                            KT[pair][hp * 64:hp * 64 + 64,
                                     ki * 128:(ki + 1) * 128],
                            qt_slice[qs][pair][hp * 64:hp * 64 + 64,
                                               o:512],
                            start=True, stop=True,
                            tile_position=(hp * 64, 0))
                    pt = ptpool.tile([128, 1024], BF16, tag="pt",
                                     name="pt")
                    if o > 0:
                        nc.gpsimd.memset(head2(pt[:], 0, o).bitcast(F32),
                                         0.0)
                    nc.scalar.activation(head2(pt[:], o, 512 - o),
                                         head2(s[:], o, 512 - o),
                                         EXPF, scale=0.125)
                    if 128 * ki >= 512 * qs:
                        for hp in range(2):
                            blk = pt[:, hp * 512 + o:hp * 512 + o + 128]
                            nc.gpsimd.affine_select(
                                out=blk, in_=blk, compare_op=GE,
                                fill=0.0, base=0, pattern=[[1, 128]],
                                channel_multiplier=-1)
                    pts[ki] = pt
                return f

            def av_mm(ki):
                def f():
                    pt = pts.pop(ki)
                    for hp in range(2):
                        h = pair * 2 + hp
                        nc.tensor.matmul(
                            avs[hp][:],
                            VP[ki][:, h * (D + 1):(h + 1) * (D + 1)],
                            pt[:, hp * 512:hp * 512 + 512],
                            start=(ki == ks), stop=(ki == ke - 1))
                return f

            def finish_hp(hp):
                if seg == "early":
                    nc.vector.tensor_copy(oacc[pair][hp][:], avs[hp][:])
                    return
                osl = ot_slice[qs][pair][hp * 64:hp * 64 + 64, :]
                rsl = rsg[qs][pair][32 * hp:32 * hp + 1, :]
                if seg == "late":
                    nc.vector.tensor_add(osl, avs[hp][0:64, :],
                                         oacc[pair][hp][0:64, :])
                    nc.vector.tensor_add(rsl, avs[hp][64:65, :],
                                         oacc[pair][hp][64:65, :])
                else:
                    nc.vector.tensor_copy(osl, avs[hp][0:64, :])
                    nc.scalar.copy(rsl, avs[hp][64:65, :])

            def op0():
                start_pair()
                s_exp(ks)()

            # AV lags S by two k-tiles: the PE only needs exp(ki-2) done
            # when it reaches av(ki-2), giving the Act engine a full tile
            # of slack (pt pool holds 4 tiles; S-psum WAR gives the same
            # two-tile depth).
            ops.append(op0)
            ops.append(s_exp(ks + 1))
            for ki in range(ks + 2, ke):
                ops.append(s_exp(ki))
                ops.append(av_mm(ki - 2))

            def last():
                av_mm(ke - 2)()
                av_mm(ke - 1)()
                for hp in range(2):
                    finish_hp(hp)

            ops.append(last)
            return ops

        def emit_att_early_ops(qs):
            """Off-diagonal k-tiles of q-slice qs (all inputs ready one
            step early); run at the end of step qs-1."""
            return (emit_att_pair_ops(qs, 0, 0, 4 * qs, "early")
                    + emit_att_pair_ops(qs, 1, 0, 4 * qs, "early"))

        def emit_att_late_ops(qs, pair):
            """Diagonal k-tiles of q-slice qs + merge of the early spill."""
            return emit_att_pair_ops(qs, pair, 4 * qs, 4 * qs + 4, "late")

        def emit_norm_pair_ops(qs, pair):
            """Reciprocal + O^T normalization for one (q-slice, pair).
            The reciprocal row is partition-broadcast on GpSimd (idle
            engine) so the PE never enters this chain."""
            ops = []

            def recip():
                nc.vector.reciprocal_approx_fast(recg[qs][pair][:],
                                                 rsg[qs][pair][:])

            def norm():
                # rb[p, q] = 1/denom(head p//64, q) via the block-indicator
                # stationary; one multiply normalizes both heads.
                rt = nrm.tile([33, 512], F32R, tag="rt", name="rt")
                nc.vector.tensor_copy(rt[:], recg[qs][pair][:])
                rb = aps.tile([128, 512], F32, tag="a", name="a")
                nc.tensor.matmul(rb[:], blk2[0:33, 0:128], rt[:],
                                 start=True, stop=True)
                sl = ot_slice[qs][pair][:]
                nc.vector.tensor_mul(sl, sl, rb[:])

            ops.append(recip)
            ops.append(norm)
            return ops

        def emit_proj_out_ops(qs, final=False):
            """c_proj tiles + store for q-slice qs (needs both pairs).
            On the final slice, the PSUM->SBUF copies alternate DVE/Act
            (Act is idle once the last exp retires) and each 512-column
            half is stored as soon as it is copied."""
            ops = []
            for i in range(qs * 4, qs * 4 + 4):
                def proj(i=i):
                    yt = ypool.tile([128, C], F32, tag="y", name="y")
                    for cs in range(2):
                        ps = aps.tile([128, 512], F32, tag="a", name="a")
                        for pair in range(PAIRS):
                            nc.tensor.matmul(
                                ps[:],
                                ot_slice[qs][pair][
                                    :, (i - qs * 4) * 128:
                                       (i - qs * 4 + 1) * 128],
                                wp_sb[pair][:, cs * 512:(cs + 1) * 512],
                                start=(pair == 0), stop=(pair == PAIRS - 1))
                        ysl = yt[:, cs * 512:(cs + 1) * 512]
                        # On the final slice both copy engines are free;
                        # alternate so neither serializes the drain.
                        if final and (i + cs) % 2 == 1:
                            nc.scalar.copy(ysl, ps[:])
                        else:
                            nc.vector.tensor_copy(ysl, ps[:])
                        nc.sync.dma_start(
                            yout[i * 128:(i + 1) * 128,
                                 cs * 512:(cs + 1) * 512], ysl)
                ops.append(proj)
            return ops

        def emit_tail_ops(qs):
            return (emit_norm_pair_ops(qs, 0) + emit_norm_pair_ops(qs, 1)
                    + emit_proj_out_ops(qs))

        # ---- fused pipeline ----
        # Prologue: slice-0 Q/K groups first (they feed everything), then
        # pair 0's attention chain woven into the V groups so the exp
        # pipeline warms up while the projections are still running.
        a0 = emit_proj_ops(0)
        for op in a0[:4]:
            op()
        w0 = Weave()
        w0.add(a0[4:])
        w0.add(emit_att_pair_ops(0, 0))
        w0.run()
        for t in range(QS):
            last_step = t == QS - 1
            w = Weave()
            if t >= 1:
                w.add(emit_tail_ops(t - 1))
            if not last_step:
                if t == 0:
                    w.add(emit_att_pair_ops(0, 1))
                else:
                    w.add(emit_att_late_ops(t, 0)
                          + emit_att_late_ops(t, 1))
                w.add(emit_proj_ops(t + 1))
                w.run()
                # off-diagonal attention for the NEXT slice: its Q/K/V all
                # exist now, so pull it ahead of its own pipeline step.
                for op in emit_att_early_ops(t + 1):
                    op()
            else:
                # final slice: only the 4 diagonal k-tiles remain; hide
                # pair 0's normalization under pair 1's attention, leaving
                # only pair 1's short tail exposed.
                w.add(emit_att_late_ops(t, 0))
                w.run()
                w2 = Weave()
                w2.add(emit_norm_pair_ops(t, 0))
                w2.add(emit_att_late_ops(t, 1))
                w2.run()
        for op in emit_norm_pair_ops(QS - 1, 1):
            op()
        for op in emit_proj_out_ops(QS - 1, final=True):
            op()


_cached_nc = None


def get_program():
    global _cached_nc
    if _cached_nc is None:
        _cached_nc = build_program()
    return _cached_nc


def kernel(x, w_attn, w_proj, _trace=False, _trace_kwargs=None):
    assert x.shape == (B, T, C) and w_attn.shape == (C, 3 * C)
    assert w_proj.shape == (C, C)
    bf16 = ml_dtypes.bfloat16
    x = np.ascontiguousarray(x, dtype=np.float32)
    w_attn = np.ascontiguousarray(w_attn, dtype=np.float32)
    w_proj = np.ascontiguousarray(w_proj, dtype=np.float32)

    xT_b = [np.ascontiguousarray(x[b].T.astype(bf16)) for b in range(B)]
    blk2_host = np.zeros((33, 128), dtype=np.float32)
    blk2_host[0, 0:64] = 1.0
    blk2_host[32, 64:128] = 1.0

    in_maps = []
    for core in range(N_CORES):
        b, g = divmod(core, 4)
        cols = slice(g * HC * D, (g + 1) * HC * D)
        wqkv = np.concatenate(
            [w_attn[:, 0:C][:, cols], w_attn[:, C:2 * C][:, cols],
             w_attn[:, 2 * C:3 * C][:, cols]], axis=1)
        in_maps.append({
            "xT": xT_b[b],
            "wqkv": np.ascontiguousarray(wqkv.astype(bf16)),
            "wp": np.ascontiguousarray(w_proj[cols, :].astype(bf16)),
            "ones16": np.ones((128, 128), dtype=bf16),
            "onesr": blk2_host,
        })

    nc = get_program()
    res = run_bass_kernel_spmd(
        nc, in_maps, list(range(N_CORES)),
        trace=_trace, **(_trace_kwargs or {}))

    y = np.zeros((B, T, C), dtype=np.float32)
    for core in range(N_CORES):
        b = core // 4
        y[b] += res.results[core]["y"].astype(np.float32)
    if _trace:
        return y, res
    return y
